# revision 19
# baseline (speedup 1.0000x reference)
# Trainium2 Bass kernel for nn_DiffNet — v9.
#
# Math (identical reduction to v5): with coef = (conv2_w @ conv1_w)[0],
# bc = conv2_w@conv1_b + conv2_b, scale = RATE/batch_num,
# C* = scale*(coef, bc), each layer reduces to
#   P = vi @ W.T + b
#   s = sum_i vi,  q = sum_i vi^2
#   alpha = 1 + C2*s,  delta = C0*q + Cb*s
#   out = alpha*relu(P) + C1*P + delta
# (C1*P carries C1*b like v5 — O(C1^2) ~ 1e-5 of the output.)
#
# Scheduling model (from trace analysis):
#  * measured time = end of the NEFF's last teardown instruction minus
#    the preamble's first memset; everything after our final output-DMA
#    enqueue is ~10us of fixed postamble. The objective is to enqueue
#    the output DMA as early as possible.
#  * All 8 cores replicate the 1.8MB fp16 weight wall, which saturates
#    device HBM at ~250-310GB/s per core aggregate across both HWDGE
#    rings; the stream is the floor, everything else must overlap it.
#  * DMA completion fires per dma_start (per-8-partition-row increments
#    cover all columns only at the end), so W1 ships as 2 quanta per
#    ring, chunk c0 with xT on ring A, c1 on ring B (both land first),
#    then c2/c3 — each L1 half {c0,c1} / {c2,c3} takes one quantum from
#    each ring and keeps o's chunk order intact.
#  * Per-half PSUM tiles let half-1 epilogues run under half-2 matmuls;
#    the bias lands via one block-diagonal matmul per half (bias rows
#    uploaded as [4,128] fp16 rows) so the relu is a single whole-half
#    activation.
#  * s/q come from ONE memset ones stationary; alpha/delta use
#    tensor_scalar float immediates.
#  * Engine split: scalar = relu + half-1 sum-squares, vector =
#    alpha/delta + t2/t3 + half-1 tail, gpsimd = half-2 tail, so the
#    chains and halves overlap.
#
# Sharding: data-parallel over batch (64 -> 8 rows/core), weights
# replicated, zero collectives. Host transposes the per-core [128,16]
# result back.

import numpy as np

RATE = 0.01
B, IN, H1, H2, OUT = 64, 1024, 512, 512, 256
NCORES = 8
BL = B // NCORES
P128 = 128

NK = [IN // P128, H1 // P128, H2 // P128]    # 8, 4, 4
NCH = [H1 // P128, H2 // P128, OUT // P128]  # 4, 4, 2

XT_LEN = NK[0] * BL  # 64

# DMA quanta per ring, in consumption order.
# ring A: [xt|W1c0] [W1c2] [W2c0|W2c2] [W3c0]
# ring B: [W1c1] [W1c3] [W2c1|W2c3] [W3c1]
A_QUANTA = [XT_LEN + 8 * P128, 8 * P128, 8 * P128, 4 * P128]
B_QUANTA = [8 * P128, 8 * P128, 8 * P128, 4 * P128]
WA_TOT = sum(A_QUANTA)  # 3648
WB_TOT = sum(B_QUANTA)  # 3584

# bias columns [128, 16] fp32: col BH_COL[l]+c = bias of chunk c
# (per-partition bias operand for the per-chunk relu activation).
BH_COL = [0, 4, 8]
BH_LEN = 16

N_WARMUP = 12

_NC_CACHE = {}
_CONSTS = {}


def _half_chunks(l, h):
    if l == 2:
        return (h,)
    return (0, 1) if h == 0 else (2, 3)


def _chunk_home(l, c):
    """-> (ring, quantum_idx, col offset within that quantum)."""
    if l == 0:
        r = c % 2           # c0,c2 -> A; c1,c3 -> B
        qi = c // 2         # c0,c1 -> first quanta; c2,c3 -> second
        off = XT_LEN if (r == 0 and qi == 0) else 0
        return r, qi, off
    if l == 1:
        r = c % 2
        return r, 2, (c // 2) * NK[1] * P128
    return (c, 3, 0)


def _build_nc():
    import concourse.bacc as bacc
    import concourse.mybir as mybir
    import concourse.tile as tile
    from concourse.bass import AP

    fp32 = mybir.dt.float32
    fp16 = mybir.dt.float16
    AF = mybir.ActivationFunctionType
    ALU = mybir.AluOpType

    nc = bacc.Bacc("TRN2", target_bir_lowering=False, debug=False)

    wa_t = nc.dram_tensor("wa", [P128, WA_TOT], fp16, kind="ExternalInput")
    wb_t = nc.dram_tensor("wb", [P128, WB_TOT], fp16, kind="ExternalInput")
    bh_t = nc.dram_tensor("bh", [P128, BH_LEN], fp32, kind="ExternalInput")
    out_t = nc.dram_tensor("outT", [P128, 2 * BL], fp32, kind="ExternalOutput")

    with tile.TileContext(nc) as tc:
        with (
            tc.tile_pool(name="wp", bufs=1) as wp,
            tc.tile_pool(name="ap", bufs=1) as ap_,
            tc.tile_pool(name="xp", bufs=1, space="PSUM") as xp,
            tc.tile_pool(name="pp", bufs=3, space="PSUM") as pp,
            tc.tile_pool(name="sp", bufs=2, space="PSUM") as sp,
            tc.tile_pool(name="qp", bufs=2, space="PSUM") as qp,
        ):
            # --- DMAs: enqueue order == transfer order per ring ---
            bhr = ap_.tile([P128, BH_LEN], fp32, tag="bhr")
            nc.scalar.dma_start(bhr[:], bh_t[:], single_packet=True)
            quanta = [[], []]  # [ring] -> list of tiles
            for r, (eng, wt, qs) in enumerate(
                ((nc.sync, wa_t, A_QUANTA), (nc.scalar, wb_t, B_QUANTA))
            ):
                lo = 0
                for qi, ncols in enumerate(qs):
                    t = wp.tile([P128, ncols], fp16, tag=f"q{r}{qi}")
                    eng.dma_start(t[:], wt[:, lo : lo + ncols])
                    quanta[r].append(t)
                    lo += ncols

            xt = quanta[0][0][:, 0:XT_LEN]

            def wchunk(l, c, k):
                r, qi, off = _chunk_home(l, c)
                lo = off + k * P128
                return quanta[r][qi][:, lo : lo + P128]

            # --- small on-device constants ---
            ones = wp.tile([P128, P128], fp16, tag="ones")
            nc.gpsimd.memset(ones[:], 1.0)
            junk_a = wp.tile([BL, BL], fp16, tag="junk_a")
            junk_w = wp.tile([BL, BL], fp16, tag="junk_w")
            nc.gpsimd.memset(junk_a[:], 0.0)
            nc.gpsimd.memset(junk_w[:], 0.0)

            # PE warm-up (HAM clock gate) while the DMAs stream
            warm = xp.tile([BL, BL], fp32, tag="warm")
            for _ in range(N_WARMUP):
                nc.tensor.matmul(warm[:], junk_a[:], junk_w[:], start=True, stop=True)

            # x^2 on vector (scalar is busy enqueueing weight DMAs)
            sq1 = ap_.tile([P128, XT_LEN], fp16, tag="sq1")
            nc.vector.tensor_tensor(sq1[:], xt, xt, ALU.mult)

            def bcast(t, n):
                """[128, 8] tile slice -> [128, n, 8] 0-stride broadcast."""
                return AP(t.tensor, t.offset, [t.ap[0], [0, n], t.ap[1]])

            C = _CONSTS  # dict: C0, C1, C2, Cb — set by host_prep

            def layer(l, viT, sq):
                nk, nch = NK[l], NCH[l]
                nh = nch // 2  # chunks per half
                hw = nh * BL   # columns per half
                last = l == 2

                bcs = sp.tile([P128, BL], fp32, tag="bcs")
                bcq = qp.tile([P128, BL], fp32, tag="bcq")

                def bcs_mms():
                    for k in range(nk):
                        nc.tensor.matmul(
                            bcs[:], ones[:], viT[:, k * BL : (k + 1) * BL],
                            start=(k == 0), stop=(k == nk - 1),
                        )

                def bcq_mms():
                    for k in range(nk):
                        nc.tensor.matmul(
                            bcq[:], ones[:], sq[:, k * BL : (k + 1) * BL],
                            start=(k == 0), stop=(k == nk - 1),
                        )

                Ph = []
                for _h in range(2):
                    Pht = pp.tile([P128, hw], fp32, tag="P")
                    Ph.append(Pht)

                def pt_mms(h):
                    for ci in range(nh):
                        c = _half_chunks(l, h)[ci]
                        for k in range(nk):
                            nc.tensor.matmul(
                                Ph[h][:, ci * BL : (ci + 1) * BL],
                                wchunk(l, c, k),
                                viT[:, k * BL : (k + 1) * BL],
                                start=(k == 0), stop=(k == nk - 1),
                            )

                # tensor-engine order: L1 does bc early (xT lands with the
                # first quantum, long before half 2); later layers Pt first.
                if l == 0:
                    bcs_mms()
                    bcq_mms()
                    pt_mms(0)
                    pt_mms(1)
                else:
                    pt_mms(0)
                    pt_mms(1)
                    bcs_mms()
                    bcq_mms()

                # alpha = 1 + C2*s; delta = C0*q + Cb*s   (vector)
                al = ap_.tile([P128, BL], fp32, tag=f"al{l}")
                nc.vector.tensor_scalar(
                    al[:], bcs[:], C["C2"], 1.0, ALU.mult, ALU.add
                )
                d1 = ap_.tile([P128, BL], fp32, tag=f"d1{l}")
                nc.vector.tensor_scalar(d1[:], bcq[:], C["C0"], None, ALU.mult)
                d2 = ap_.tile([P128, BL], fp32, tag=f"d2{l}")
                nc.vector.tensor_scalar(d2[:], bcs[:], C["Cb"], None, ALU.mult)
                dl = ap_.tile([P128, BL], fp32, tag=f"dl{l}")
                nc.vector.tensor_tensor(dl[:], d1[:], d2[:], ALU.add)

                o = ap_.tile([P128, nch * BL], fp32 if last else fp16, tag=f"o{l}")
                sqn = (
                    None if last
                    else ap_.tile([P128, nch * BL], fp16, tag=f"sqn{l}")
                )
                for h in range(2):
                    # per-chunk relu with per-partition bias (scalar)
                    R = ap_.tile([P128, hw], fp32, tag=f"R{l}{h}")
                    for ci in range(nh):
                        c = _half_chunks(l, h)[ci]
                        col = BH_COL[l] + c
                        nc.scalar.activation(
                            out=R[:, ci * BL : (ci + 1) * BL],
                            in_=Ph[h][:, ci * BL : (ci + 1) * BL],
                            func=AF.Relu,
                            bias=bhr[:, col : col + 1],
                        )
                    t2 = ap_.tile([P128, hw], fp32, tag=f"t2{l}{h}")
                    nc.vector.tensor_scalar(t2[:], Ph[h][:], C["C1"], None, ALU.mult)
                    t3 = ap_.tile([P128, hw], fp32, tag=f"t3{l}{h}")
                    nc.vector.tensor_tensor(t3[:], t2[:], bcast(dl, nh), ALU.add)
                    t4 = ap_.tile([P128, hw], fp32, tag=f"t4{l}{h}")
                    oh = o[:, h * hw : (h + 1) * hw]
                    if h == 0:
                        nc.vector.tensor_tensor(t4[:], R[:], bcast(al, nh), ALU.mult)
                        nc.vector.tensor_tensor(oh, t3[:], t4[:], ALU.add)
                    else:
                        nc.gpsimd.tensor_tensor(t4[:], R[:], bcast(al, nh), ALU.mult)
                        nc.gpsimd.tensor_tensor(oh, t3[:], t4[:], ALU.add)
                    if last:
                        eng = nc.scalar if h == 0 else nc.sync
                        eng.dma_start(
                            out_t[:, h * hw : (h + 1) * hw], oh, single_packet=True
                        )
                    else:
                        if h == 0:
                            nc.scalar.activation(
                                out=sqn[:, 0:hw], in_=oh, func=AF.Square
                            )
                        else:
                            nc.gpsimd.tensor_tensor(
                                sqn[:, hw : 2 * hw], oh, oh, ALU.mult
                            )
                return o, sqn

            o1, sq2 = layer(0, xt, sq1[:])
            o2, sq3 = layer(1, o1[:], sq2[:])
            layer(2, o2[:], sq3[:])

    nc.compile()
    return nc


def get_nc():
    assert _CONSTS, "call host_prep() before get_nc() — constants are baked in"
    key = tuple(sorted(_CONSTS.items()))
    if _NC_CACHE.get("key") != key:
        _NC_CACHE["nc"] = _build_nc()
        _NC_CACHE["key"] = key
    return _NC_CACHE["nc"]


def host_prep(x, fc1_w, fc1_b, fc2_w, fc2_b, fc3_w, fc3_b,
              conv1_w, conv1_b, conv2_w, conv2_b, batch_num):
    f32, f16, f64 = np.float32, np.float16, np.float64
    x = np.asarray(x, f32)
    ws = [np.asarray(fc1_w, f32), np.asarray(fc2_w, f32), np.asarray(fc3_w, f32)]
    bs = [np.asarray(fc1_b, f32), np.asarray(fc2_b, f32), np.asarray(fc3_b, f32)]

    bn = float(np.asarray(batch_num).item())
    scale = RATE / bn
    coef = (np.asarray(conv2_w, f64) @ np.asarray(conv1_w, f64))[0]
    bcv = float(
        (np.asarray(conv2_w, f64) @ np.asarray(conv1_b, f64))[0]
        + np.asarray(conv2_b, f64)[0]
    )
    C0, C1, C2 = (scale * coef).astype(f64)
    Cb = scale * bcv
    _CONSTS.clear()
    _CONSTS.update(
        {"C0": float(C0), "C1": float(C1), "C2": float(C2), "Cb": float(Cb)}
    )

    bh = np.zeros((P128, BH_LEN), f32)
    for l in range(3):
        for c in range(NCH[l]):
            bh[:, BH_COL[l] + c] = bs[l][c * P128 : (c + 1) * P128]

    wa_base = np.zeros((P128, WA_TOT), f16)
    wb_base = np.zeros((P128, WB_TOT), f16)
    a_off = [0]
    for q in A_QUANTA:
        a_off.append(a_off[-1] + q)
    b_off = [0]
    for q in B_QUANTA:
        b_off.append(b_off[-1] + q)

    for l in range(3):
        Wt = ws[l].T.astype(f16)  # [in, out]
        nk = NK[l]
        for c in range(NCH[l]):
            r, qi, off = _chunk_home(l, c)
            dst = wa_base if r == 0 else wb_base
            base = (a_off if r == 0 else b_off)[qi] + off
            for k in range(nk):
                chunk = Wt[k * P128 : (k + 1) * P128, c * P128 : (c + 1) * P128]
                dst[:, base + k * P128 : base + (k + 1) * P128] = chunk

    in_maps = []
    for kcore in range(NCORES):
        xk = x[kcore * BL : (kcore + 1) * BL]
        xt = (
            xk.T.reshape(NK[0], P128, BL).transpose(1, 0, 2).reshape(P128, XT_LEN)
        ).astype(f16)
        wa = wa_base.copy()
        wa[:, 0:XT_LEN] = xt
        in_maps.append({"wa": wa, "wb": wb_base, "bh": bh})
    return in_maps


def _unshard(outT):
    """[128, 16] -> [8, 256]: out[b, c*128+p] = outT[p, c*8+b]."""
    return np.ascontiguousarray(
        outT.reshape(P128, 2, BL).transpose(2, 1, 0).reshape(BL, OUT), dtype=np.float32
    )


def kernel(**inputs):
    from concourse.bass_utils import run_bass_kernel_spmd

    in_maps = host_prep(**inputs)
    nc = get_nc()
    res = run_bass_kernel_spmd(nc, in_maps, core_ids=list(range(NCORES)))
    out = np.concatenate(
        [_unshard(res.results[k]["outT"]) for k in range(NCORES)], axis=0
    )
    return np.ascontiguousarray(out, dtype=np.float32)


# revision 20
# speedup vs baseline: 1.0432x; 1.0432x over previous
# Trainium2 Bass kernel for nn_DiffNet — v9.
#
# Math (identical reduction to v5): with coef = (conv2_w @ conv1_w)[0],
# bc = conv2_w@conv1_b + conv2_b, scale = RATE/batch_num,
# C* = scale*(coef, bc), each layer reduces to
#   P = vi @ W.T + b
#   s = sum_i vi,  q = sum_i vi^2
#   alpha = 1 + C2*s,  delta = C0*q + Cb*s
#   out = alpha*relu(P) + C1*P + delta
# (C1*P carries C1*b like v5 — O(C1^2) ~ 1e-5 of the output.)
#
# Scheduling model (from trace analysis):
#  * measured time = end of the NEFF's last teardown instruction minus
#    the preamble's first memset; everything after our final output-DMA
#    enqueue is ~10us of fixed postamble. The objective is to enqueue
#    the output DMA as early as possible.
#  * All 8 cores replicate the 1.8MB fp16 weight wall, which saturates
#    device HBM at ~250-310GB/s per core aggregate across both HWDGE
#    rings; the stream is the floor, everything else must overlap it.
#  * DMA completion fires per dma_start (per-8-partition-row increments
#    cover all columns only at the end), so W1 ships as 2 quanta per
#    ring, chunk c0 with xT on ring A, c1 on ring B (both land first),
#    then c2/c3 — each L1 half {c0,c1} / {c2,c3} takes one quantum from
#    each ring and keeps o's chunk order intact.
#  * Per-half PSUM tiles let half-1 epilogues run under half-2 matmuls;
#    the bias lands via one block-diagonal matmul per half (bias rows
#    uploaded as [4,128] fp16 rows) so the relu is a single whole-half
#    activation.
#  * s/q come from ONE memset ones stationary; alpha/delta use
#    tensor_scalar float immediates.
#  * Engine split: scalar = relu + half-1 sum-squares, vector =
#    alpha/delta + t2/t3 + half-1 tail, gpsimd = half-2 tail, so the
#    chains and halves overlap.
#
# Sharding: data-parallel over batch (64 -> 8 rows/core), weights
# replicated, zero collectives. Host transposes the per-core [128,16]
# result back.

import numpy as np

RATE = 0.01
B, IN, H1, H2, OUT = 64, 1024, 512, 512, 256
NCORES = 8
BL = B // NCORES
P128 = 128

NK = [IN // P128, H1 // P128, H2 // P128]    # 8, 4, 4
NCH = [H1 // P128, H2 // P128, OUT // P128]  # 4, 4, 2

XT_LEN = NK[0] * BL  # 64

# DMA quanta per ring, in consumption order (3 rings: sync HWDGE,
# scalar HWDGE, gpsimd SWDGE — the software ring has ~2.5us extra
# startup latency, so it carries W3, needed last).
# ring A (sync):   [xt|W1c0] [W1c2] [W2c0|W2c2]
# ring B (scalar): [W1c1] [W1c3] [W2c1|W2c3]
# ring G (gpsimd): [W3c0|W3c1]
A_QUANTA = [XT_LEN + 8 * P128, 8 * P128, 8 * P128]
B_QUANTA = [8 * P128, 8 * P128, 8 * P128]
WA_TOT = sum(A_QUANTA)  # 3136
WB_TOT = sum(B_QUANTA)  # 3072
WG_TOT = 8 * P128       # 1024

# bias rows [1, 10*128+8] fp16 on partition 0: block BR_IDX[l]+c is the
# layer-l chunk-c bias as a row; the rank-1 matmul
# (bias_row)^T @ ones8 opens each chunk's PSUM accumulation group.
BR_IDX = [0, 4, 8]
ONES_OFF = 10 * P128
BR_LEN = 10 * P128 + BL

N_WARMUP = 12

_NC_CACHE = {}
_CONSTS = {}


def _half_chunks(l, h):
    if l == 2:
        return (h,)
    return (0, 1) if h == 0 else (2, 3)


def _chunk_home(l, c):
    """-> (ring, quantum_idx, col offset within that quantum)."""
    if l == 0:
        r = c % 2           # c0,c2 -> A; c1,c3 -> B
        qi = c // 2         # c0,c1 -> first quanta; c2,c3 -> second
        off = XT_LEN if (r == 0 and qi == 0) else 0
        return r, qi, off
    if l == 1:
        r = c % 2
        return r, 2, (c // 2) * NK[1] * P128
    return (2, 0, c * NK[2] * P128)  # ring G


def _build_nc():
    import concourse.bacc as bacc
    import concourse.mybir as mybir
    import concourse.tile as tile
    from concourse.bass import AP

    fp32 = mybir.dt.float32
    fp16 = mybir.dt.float16
    AF = mybir.ActivationFunctionType
    ALU = mybir.AluOpType

    nc = bacc.Bacc("TRN2", target_bir_lowering=False, debug=False)

    wa_t = nc.dram_tensor("wa", [P128, WA_TOT], fp16, kind="ExternalInput")
    wb_t = nc.dram_tensor("wb", [P128, WB_TOT], fp16, kind="ExternalInput")
    wg_t = nc.dram_tensor("wg", [P128, WG_TOT], fp16, kind="ExternalInput")
    br_t = nc.dram_tensor("br", [1, BR_LEN], fp16, kind="ExternalInput")
    out_t = nc.dram_tensor("outT", [P128, 2 * BL], fp32, kind="ExternalOutput")

    with tile.TileContext(nc) as tc:
        with (
            tc.tile_pool(name="wp", bufs=1) as wp,
            tc.tile_pool(name="ap", bufs=1) as ap_,
            tc.tile_pool(name="xp", bufs=1, space="PSUM") as xp,
            tc.tile_pool(name="pp", bufs=3, space="PSUM") as pp,
            tc.tile_pool(name="sp", bufs=2, space="PSUM") as sp,
            tc.tile_pool(name="qp", bufs=2, space="PSUM") as qp,
        ):
            # --- DMAs: enqueue order == transfer order per ring ---
            brr = ap_.tile([1, BR_LEN], fp16, tag="brr")
            nc.scalar.dma_start(brr[:], br_t[:], single_packet=True)
            quanta = [[], [], []]  # [ring] -> list of tiles
            for r, (eng, wt, qs) in enumerate(
                (
                    (nc.sync, wa_t, A_QUANTA),
                    (nc.scalar, wb_t, B_QUANTA),
                    (nc.gpsimd, wg_t, [WG_TOT]),
                )
            ):
                lo = 0
                for qi, ncols in enumerate(qs):
                    t = wp.tile([P128, ncols], fp16, tag=f"q{r}{qi}")
                    eng.dma_start(t[:], wt[:, lo : lo + ncols])
                    quanta[r].append(t)
                    lo += ncols

            xt = quanta[0][0][:, 0:XT_LEN]

            def wchunk(l, c, k):
                r, qi, off = _chunk_home(l, c)
                lo = off + k * P128
                return quanta[r][qi][:, lo : lo + P128]

            # --- small on-device constants ---
            ones = wp.tile([P128, P128], fp16, tag="ones")
            nc.gpsimd.memset(ones[:], 1.0)
            junk_a = wp.tile([BL, BL], fp16, tag="junk_a")
            junk_w = wp.tile([BL, BL], fp16, tag="junk_w")
            nc.gpsimd.memset(junk_a[:], 0.0)
            nc.gpsimd.memset(junk_w[:], 0.0)

            # PE warm-up (HAM clock gate) while the DMAs stream
            warm = xp.tile([BL, BL], fp32, tag="warm")
            for _ in range(N_WARMUP):
                nc.tensor.matmul(warm[:], junk_a[:], junk_w[:], start=True, stop=True)

            # x^2 on vector (scalar is busy enqueueing weight DMAs)
            sq1 = ap_.tile([P128, XT_LEN], fp16, tag="sq1")
            nc.vector.tensor_tensor(sq1[:], xt, xt, ALU.mult)

            def bcast(t, n):
                """[128, 8] tile slice -> [128, n, 8] 0-stride broadcast."""
                return AP(t.tensor, t.offset, [t.ap[0], [0, n], t.ap[1]])

            C = _CONSTS  # dict: C0, C1, C2, Cb — set by host_prep

            def layer(l, viT, sq):
                nk, nch = NK[l], NCH[l]
                nh = nch // 2  # chunks per half
                hw = nh * BL   # columns per half
                last = l == 2

                bcs = sp.tile([P128, BL], fp32, tag="bcs")
                bcq = qp.tile([P128, BL], fp32, tag="bcq")

                def bcs_mms():
                    for k in range(nk):
                        nc.tensor.matmul(
                            bcs[:], ones[:], viT[:, k * BL : (k + 1) * BL],
                            start=(k == 0), stop=(k == nk - 1),
                        )

                def bcq_mms():
                    for k in range(nk):
                        nc.tensor.matmul(
                            bcq[:], ones[:], sq[:, k * BL : (k + 1) * BL],
                            start=(k == 0), stop=(k == nk - 1),
                        )

                Ph = []
                for _h in range(2):
                    Pht = pp.tile([P128, hw], fp32, tag="P")
                    Ph.append(Pht)

                def pt_mms(h):
                    for ci in range(nh):
                        c = _half_chunks(l, h)[ci]
                        blk = BR_IDX[l] + c
                        reg = Ph[h][:, ci * BL : (ci + 1) * BL]
                        # rank-1 bias opens the accumulation group
                        nc.tensor.matmul(
                            reg,
                            brr[0:1, blk * P128 : (blk + 1) * P128],
                            brr[0:1, ONES_OFF : ONES_OFF + BL],
                            start=True, stop=False,
                        )
                        for k in range(nk):
                            nc.tensor.matmul(
                                reg,
                                wchunk(l, c, k),
                                viT[:, k * BL : (k + 1) * BL],
                                start=False, stop=(k == nk - 1),
                            )

                # tensor-engine order: L1 does bc early (xT lands with the
                # first quantum, long before half 2); later layers Pt first.
                if l == 0:
                    bcs_mms()
                    bcq_mms()
                    pt_mms(0)
                    pt_mms(1)
                else:
                    pt_mms(0)
                    pt_mms(1)
                    bcs_mms()
                    bcq_mms()

                # alpha = 1 + C2*s; delta = C0*q + Cb*s   (vector)
                al = ap_.tile([P128, BL], fp32, tag=f"al{l}")
                nc.vector.tensor_scalar(
                    al[:], bcs[:], C["C2"], 1.0, ALU.mult, ALU.add
                )
                d1 = ap_.tile([P128, BL], fp32, tag=f"d1{l}")
                nc.vector.tensor_scalar(d1[:], bcq[:], C["C0"], None, ALU.mult)
                d2 = ap_.tile([P128, BL], fp32, tag=f"d2{l}")
                nc.vector.tensor_scalar(d2[:], bcs[:], C["Cb"], None, ALU.mult)
                dl = ap_.tile([P128, BL], fp32, tag=f"dl{l}")
                nc.vector.tensor_tensor(dl[:], d1[:], d2[:], ALU.add)

                o = ap_.tile([P128, nch * BL], fp32 if last else fp16, tag=f"o{l}")
                sqn = (
                    None if last
                    else ap_.tile([P128, nch * BL], fp16, tag=f"sqn{l}")
                )
                for h in range(2):
                    # whole-half relu (bias already in PSUM via rank-1)
                    R = ap_.tile([P128, hw], fp32, tag=f"R{l}{h}")
                    nc.scalar.activation(out=R[:], in_=Ph[h][:], func=AF.Relu)
                    t2 = ap_.tile([P128, hw], fp32, tag=f"t2{l}{h}")
                    nc.vector.tensor_scalar(t2[:], Ph[h][:], C["C1"], None, ALU.mult)
                    t3 = ap_.tile([P128, hw], fp32, tag=f"t3{l}{h}")
                    nc.vector.tensor_tensor(t3[:], t2[:], bcast(dl, nh), ALU.add)
                    t4 = ap_.tile([P128, hw], fp32, tag=f"t4{l}{h}")
                    oh = o[:, h * hw : (h + 1) * hw]
                    if h == 0:
                        nc.vector.tensor_tensor(t4[:], R[:], bcast(al, nh), ALU.mult)
                        nc.vector.tensor_tensor(oh, t3[:], t4[:], ALU.add)
                    else:
                        nc.gpsimd.tensor_tensor(t4[:], R[:], bcast(al, nh), ALU.mult)
                        nc.gpsimd.tensor_tensor(oh, t3[:], t4[:], ALU.add)
                    if last:
                        eng = nc.scalar if h == 0 else nc.sync
                        eng.dma_start(
                            out_t[:, h * hw : (h + 1) * hw], oh, single_packet=True
                        )
                    else:
                        if h == 0:
                            nc.scalar.activation(
                                out=sqn[:, 0:hw], in_=oh, func=AF.Square
                            )
                        else:
                            nc.gpsimd.tensor_tensor(
                                sqn[:, hw : 2 * hw], oh, oh, ALU.mult
                            )
                return o, sqn

            o1, sq2 = layer(0, xt, sq1[:])
            o2, sq3 = layer(1, o1[:], sq2[:])
            layer(2, o2[:], sq3[:])

    nc.compile()
    return nc


def get_nc():
    assert _CONSTS, "call host_prep() before get_nc() — constants are baked in"
    key = tuple(sorted(_CONSTS.items()))
    if _NC_CACHE.get("key") != key:
        _NC_CACHE["nc"] = _build_nc()
        _NC_CACHE["key"] = key
    return _NC_CACHE["nc"]


def host_prep(x, fc1_w, fc1_b, fc2_w, fc2_b, fc3_w, fc3_b,
              conv1_w, conv1_b, conv2_w, conv2_b, batch_num):
    f32, f16, f64 = np.float32, np.float16, np.float64
    x = np.asarray(x, f32)
    ws = [np.asarray(fc1_w, f32), np.asarray(fc2_w, f32), np.asarray(fc3_w, f32)]
    bs = [np.asarray(fc1_b, f32), np.asarray(fc2_b, f32), np.asarray(fc3_b, f32)]

    bn = float(np.asarray(batch_num).item())
    scale = RATE / bn
    coef = (np.asarray(conv2_w, f64) @ np.asarray(conv1_w, f64))[0]
    bcv = float(
        (np.asarray(conv2_w, f64) @ np.asarray(conv1_b, f64))[0]
        + np.asarray(conv2_b, f64)[0]
    )
    C0, C1, C2 = (scale * coef).astype(f64)
    Cb = scale * bcv
    _CONSTS.clear()
    _CONSTS.update(
        {"C0": float(C0), "C1": float(C1), "C2": float(C2), "Cb": float(Cb)}
    )

    br = np.zeros((1, BR_LEN), f16)
    for l in range(3):
        for c in range(NCH[l]):
            blk = BR_IDX[l] + c
            br[0, blk * P128 : (blk + 1) * P128] = bs[l][c * P128 : (c + 1) * P128]
    br[0, ONES_OFF : ONES_OFF + BL] = 1.0

    wa_base = np.zeros((P128, WA_TOT), f16)
    wb_base = np.zeros((P128, WB_TOT), f16)
    wg_base = np.zeros((P128, WG_TOT), f16)
    walls = (wa_base, wb_base, wg_base)
    a_off = [0]
    for q in A_QUANTA:
        a_off.append(a_off[-1] + q)
    b_off = [0]
    for q in B_QUANTA:
        b_off.append(b_off[-1] + q)
    offs = (a_off, b_off, [0])

    for l in range(3):
        Wt = ws[l].T.astype(f16)  # [in, out]
        nk = NK[l]
        for c in range(NCH[l]):
            r, qi, off = _chunk_home(l, c)
            dst = walls[r]
            base = offs[r][qi] + off
            for k in range(nk):
                chunk = Wt[k * P128 : (k + 1) * P128, c * P128 : (c + 1) * P128]
                dst[:, base + k * P128 : base + (k + 1) * P128] = chunk

    in_maps = []
    for kcore in range(NCORES):
        xk = x[kcore * BL : (kcore + 1) * BL]
        xt = (
            xk.T.reshape(NK[0], P128, BL).transpose(1, 0, 2).reshape(P128, XT_LEN)
        ).astype(f16)
        wa = wa_base.copy()
        wa[:, 0:XT_LEN] = xt
        in_maps.append({"wa": wa, "wb": wb_base, "wg": wg_base, "br": br})
    return in_maps


def _unshard(outT):
    """[128, 16] -> [8, 256]: out[b, c*128+p] = outT[p, c*8+b]."""
    return np.ascontiguousarray(
        outT.reshape(P128, 2, BL).transpose(2, 1, 0).reshape(BL, OUT), dtype=np.float32
    )


def kernel(**inputs):
    from concourse.bass_utils import run_bass_kernel_spmd

    in_maps = host_prep(**inputs)
    nc = get_nc()
    res = run_bass_kernel_spmd(nc, in_maps, core_ids=list(range(NCORES)))
    out = np.concatenate(
        [_unshard(res.results[k]["outT"]) for k in range(NCORES)], axis=0
    )
    return np.ascontiguousarray(out, dtype=np.float32)
